# revision 8
# baseline (speedup 1.0000x reference)
"""Trainium2 Bass kernel for nn_BClassifier (spiking MLP classifier), v2/v3.

Data-parallel over batch: 128 samples -> 16 per NeuronCore (8 cores).
HBM-stack partner cores (2c, 2c+1) split the F=12288 contraction in half
(pair k-split); each core computes partial h for BOTH batches of the pair
and a per-quad (4 hidden tiles) ReduceScatter(add) completes h.

v2 (MM_MODE=f32r): fc1 in float32r, bit-exact h.
v3 (MM_MODE=fp8dr): fc1 via fp8e4m3 DoubleRow matmuls (0.5 cyc/row, 2
  k-planes per instruction). Three-term decomposition at native scale:
    h*SX*SW = xh@wh + xh@w2 + x2@w3
  with xh=q8(x*SX), x2=q8(q8((x*SX-xh)*16)/4) ~= 4*(x*SX-xh),
  wh=q8(W.T*SW), w2=q8(q8((W.T*SW-wh)*16)/16) ~= W.T*SW-wh, w3=q8(wh/4).
  All three terms accumulate into one PSUM; the drain applies 1/(SX*SW).

Structure for engine overlap:
  - 4 j-quads; per quad: k-outer matmul loop into 8 PSUM banks, drain with
    bias (even core) in next-use order, DMA to DRAM bounce, per-quad
    ReduceScatter overlapped with the next quad's matmuls, readback, then
    the DVE hidden-LIF scan for that quad.
  - The output-layer matmul is split by j-quad and emitted one quad late so
    the PE queue never waits on DVE. The tiny memo scan runs at the end,
    pipelined with the last quad's hidden scan in T-groups.

Infrastructure note: this walrus build accepts only ONE sync wait per
instruction; _legalize_waits splits Tile's multi-waits onto NoOps.
"""

import os
import sys

import numpy as np

sys.path.insert(0, "/opt/trn_rl_repo")

B, T, C, HH, WW = 128, 25, 3, 64, 64
F = C * HH * WW            # 12288
HID, O = 2048, 2
NCORES = 8
BL = B // NCORES           # 16 samples per core
N = T * BL                 # 400 matmul moving columns per batch group
KT = F // 128              # 96 contraction tiles
KH = KT // 2               # 48 k-tiles per core (pair k-split)
JT = HID // 128            # 16 hidden tiles
NW = 2 * N                 # both batches' columns
BETA = 0.9
THR = 1.0
TG = 5                     # timesteps per scan/omm/memo group
NG = T // TG

MM_MODE = os.environ.get("MM_MODE", "fp8dr")
SW = float(os.environ.get("FP8_SW", "128"))
SX = float(os.environ.get("FP8_SX", "1.41"))

_cache = {}


def _legalize_waits(nc, mybir):
    """Split multi-waits onto standalone NoOps (single wait slot per inst)."""
    import bass_rust

    n = 0
    for f in nc.m.functions:
        new_blocks = []
        changed = False
        for bb in f.blocks:
            out = []
            for inst in bb.instructions:
                si = inst.sync_info
                if si and len(si.on_wait) > 1:
                    changed = True
                    waits = list(si.on_wait)
                    for w in waits[:-1]:
                        n += 1
                        out.append(mybir.InstNoOp(
                            name=f"WSPLIT-{n}",
                            engine=inst.engine,
                            ins=[], outs=[],
                            sync_info=mybir.SyncInfo(on_wait=[w], on_update=[]),
                        ))
                    inst.sync_info = mybir.SyncInfo(
                        on_wait=[waits[-1]], on_update=list(si.on_update))
                out.append(inst)
            new_blocks.append(bass_rust.BasicBlock(
                name=bb.name, instructions=out,
                IsPredicated=bb.IsPredicated, IsExit=bb.IsExit,
                IsLoopEntry=bb.IsLoopEntry,
            ))
        if changed:
            f.blocks = new_blocks


def _build(mode):
    import concourse.bass as bass
    import concourse.tile as tile
    from concourse import mybir
    from contextlib import ExitStack

    f32 = mybir.dt.float32
    Alu = mybir.AluOpType
    Act = mybir.ActivationFunctionType

    fp8 = mode == "fp8dr"
    if fp8:
        mm_dt = mybir.dt.float8e4
        NR = 3                 # weight/x regions per k-tile
        WKC = 4                # k-tiles per W chunk (even: DoubleRow pairs)
        DSC = 1.0 / (SX * SW)  # drain scale
    else:
        mm_dt = {"f32": f32, "f32r": mybir.dt.float32r}[MM_MODE]
        NR = 1
        WKC = 3
        DSC = 1.0
    NKC = KH // WKC            # W chunks per quad

    nc = bass.Bass("TRN2", target_bir_lowering=False, debug=False,
                   num_devices=NCORES)
    if fp8:
        xh_d = nc.dram_tensor("xh8", [KH * 128, NW], mm_dt, kind="ExternalInput").ap()
        xl_d = nc.dram_tensor("xl8", [KH * 128, NW], mm_dt, kind="ExternalInput").ap()
        w_d = nc.dram_tensor("w8", [KH * NR * 128, HID], mm_dt, kind="ExternalInput").ap()
    else:
        xh_d = nc.dram_tensor("xt2b", [KH * 128, NW], mm_dt, kind="ExternalInput").ap()
        xl_d = None
        w_d = nc.dram_tensor("w1th", [KH * 128, HID], mm_dt, kind="ExternalInput").ap()
    b1_d = nc.dram_tensor("b1c", [128, JT], f32, kind="ExternalInput").ap()
    wot_d = nc.dram_tensor("wot", [128, O * JT], f32, kind="ExternalInput").ap()
    bo_d = nc.dram_tensor("bo2", [O, 1], f32, kind="ExternalInput").ap()
    out_d = nc.dram_tensor("out", [O, BL], f32, kind="ExternalOutput").ap()

    with tile.TileContext(nc) as tc, ExitStack() as ctx:
        const_p = ctx.enter_context(tc.tile_pool(name="const", bufs=1))
        xt_p = ctx.enter_context(tc.tile_pool(name="xt", bufs=1))
        w_p = ctx.enter_context(tc.tile_pool(name="w", bufs=4 if fp8 else 3))
        h_p = ctx.enter_context(tc.tile_pool(name="h", bufs=1))
        st_p = ctx.enter_context(tc.tile_pool(name="st", bufs=2))
        ps_p = ctx.enter_context(tc.tile_pool(name="ps", bufs=8, space="PSUM"))

        sm_p = ctx.enter_context(tc.tile_pool(name="sm", bufs=1))
        dram_p = ctx.enter_context(tc.tile_pool(name="dram", bufs=1, space="DRAM"))

        b1_sb = const_p.tile([128, JT], f32)
        wot_sb = const_p.tile([128, O * JT], f32)
        bo_sb = const_p.tile([O, 1], f32)

        def load_consts():
            nc.scalar.dma_start(b1_sb[:, :], b1_d)
            nc.scalar.dma_start(wot_sb[:, :], wot_d)
            nc.scalar.dma_start(bo_sb[:, :], bo_d)

        # x resident in SBUF: per region [128, KH*NW], col = k*NW + cg*N + tb
        xh_sb = xt_p.tile([128, KH * NW], mm_dt)
        xh_r = xh_d.rearrange("(k p) n -> p k n", p=128)
        if fp8:
            xl_sb = xt_p.tile([128, KH * NW], mm_dt)
            xl_r = xl_d.rearrange("(k p) n -> p k n", p=128)

        XCH = 2    # x chunk granularity (k-tiles): small, to not delay W

        def load_x_chunk(ck):
            k0, k1 = ck * XCH, min((ck + 1) * XCH, KH)
            nc.sync.dma_start(
                xh_sb[:, k0 * NW:k1 * NW].rearrange("p (k n) -> p k n", n=NW),
                xh_r[:, k0:k1, :])
            if fp8:
                nc.sync.dma_start(
                    xl_sb[:, k0 * NW:k1 * NW].rearrange("p (k n) -> p k n", n=NW),
                    xl_r[:, k0:k1, :])

        NXC = (KH + XCH - 1) // XCH

        # final h (then spikes in place): [128, 6400], col = j*400 + t*16 + b
        h_all = h_p.tile([128, JT * T * BL], f32)

        # W DRAM view: [(k r p) h] -> [p, k, r, h-slice]
        w_r = w_d.rearrange("(k r p) h -> p k r h", r=NR, p=128)

        # per-quad bounce buffers for the ReduceScatter
        in_b = [dram_p.tile([2 * 4 * 128, N], f32, name=f"in_b{q}")
                for q in range(4)]
        out_b = [dram_p.tile([4 * 128, N], f32, name=f"out_b{q}")
                 for q in range(4)]

        # ---- phase 2-4 helpers -------------------------------------------
        mem1 = sm_p.tile([128, JT * BL], f32)
        o_sb = sm_p.tile([O, N], f32)
        memo = sm_p.tile([O, BL], f32)
        so_all = sm_p.tile([O, N], f32)
        h4 = h_all[:, :].rearrange("p (g t b) -> p g t b", g=JT, t=T)
        ot = lambda t: o_sb[:, t * BL:(t + 1) * BL]
        st = lambda t: so_all[:, t * BL:(t + 1) * BL]
        def scan_group(g, j0, j1):
            m = mem1[:, j0 * BL:j1 * BL]
            ht = lambda t: h4[:, j0:j1, t, :]
            for t in range(TG * g, TG * (g + 1)):
                if t == 0:
                    nc.vector.tensor_copy(m, ht(0))
                else:
                    nc.vector.scalar_tensor_tensor(
                        m, m, BETA, ht(t), Alu.mult, Alu.add)
                    nc.vector.tensor_tensor(m, m, ht(t - 1), Alu.subtract)
                nc.vector.tensor_scalar(ht(t), m, THR, None, Alu.is_gt)

        def omm_group(g):
            # output-layer matmul for T-group g over all 16 j-tiles
            pog = ps_p.tile([O, TG * BL], f32, name=f"po_{g}", tag="pscg")
            for j in range(JT):
                nc.tensor.matmul(
                    pog[:, :],
                    lhsT=wot_sb[:, O * j:O * (j + 1)],
                    rhs=h_all[:, j * N + g * TG * BL:j * N + (g + 1) * TG * BL],
                    start=(j == 0),
                    stop=(j == JT - 1),
                )
            return pog

        def memo_group(g, pog):
            nc.vector.tensor_scalar(
                o_sb[:, g * TG * BL:(g + 1) * TG * BL],
                pog[:, :], bo_sb[:, 0:1], None, Alu.add)
            for t in range(TG * g, TG * (g + 1)):
                if t == 0:
                    nc.vector.tensor_copy(memo[:, :], ot(0))
                else:
                    nc.vector.scalar_tensor_tensor(
                        memo[:, :], memo[:, :], BETA, ot(t), Alu.mult, Alu.add)
                    nc.vector.tensor_tensor(
                        memo[:, :], memo[:, :], st(t - 1), Alu.subtract)
                nc.vector.tensor_scalar(st(t), memo[:, :], THR, None, Alu.is_gt)

        # ---- phase 1: 4 j-quads of fc1 -----------------------------------
        # Global W-chunk pipeline: prefetch PF chunks ahead (crossing quad
        # boundaries so the bounce DMAs never sit in front of the W stream),
        # with x pieces interleaved just ahead of their first use.
        PF = 3 if fp8 else 2
        chunks = [(q, kc) for q in range(4) for kc in range(NKC)]
        w_tiles = {}
        x_emitted = 0
        consts_loaded = [False]

        def emit_chunk_dma(ci):
            nonlocal x_emitted
            q, kc = chunks[ci]
            # x pieces stay just ahead of the matmul k-position
            if q == 0:
                want = min(NXC, ((kc + 1) * WKC + XCH - 1) // XCH)
                while x_emitted < want:
                    load_x_chunk(x_emitted)
                    x_emitted += 1
            wt = w_p.tile([128, WKC * NR * 512], mm_dt)
            dma_eng = nc.sync
            dma_eng.dma_start(
                wt[:, :].rearrange("p (k r c) -> p k r c", k=WKC, r=NR),
                w_r[:, kc * WKC:(kc + 1) * WKC, :, q * 512:(q + 1) * 512],
            )
            w_tiles[ci] = wt
            if not consts_loaded[0]:
                consts_loaded[0] = True
                load_consts()

        for ci in range(PF):
            emit_chunk_dma(ci)

        for q in range(4):
            ps_cg = [ps_p.tile([128, N], f32, name=f"ps_{q}_{i}", tag="pscg")
                     for i in range(8)]  # index jq*2+cg, issue order
            for kc in range(NKC):
                ci = q * NKC + kc
                if ci + PF < len(chunks):
                    emit_chunk_dma(ci + PF)
                wt = w_tiles.pop(ci)
                wv = wt[:, :].rearrange("p (k r c) -> p k r c", k=WKC, r=NR)
                # first chunk of a quad runs bank-major so the previous
                # quad's PSUM drains are awaited incrementally, not all at
                # once; later chunks run k-major (order within a chunk is
                # free — accumulation is per-bank).
                if fp8:
                    xh_v = xh_sb[:, :].rearrange("p (k c n) -> p k c n", k=KH, c=2)
                    xl_v = xl_sb[:, :].rearrange("p (k c n) -> p k c n", k=KH, c=2)

                    def emit_bank8(jq, cg, kp):
                        k = kc * WKC + 2 * kp
                        kl = 2 * kp
                        pt = ps_cg[jq * 2 + cg]
                        for r, xv in ((0, xh_v), (1, xh_v), (2, xl_v)):
                            nc.tensor.matmul(
                                pt[:, :],
                                lhsT=wv[:, kl:kl + 2, r, jq * 128:(jq + 1) * 128],
                                rhs=xv[:, k:k + 2, cg, :],
                                start=(k == 0 and r == 0),
                                stop=(k == KH - 2 and r == 2),
                                perf_mode=mybir.MatmulPerfMode.DoubleRow,
                            )

                    if (kc == 0 and q > 0) or kc == NKC - 1:
                        for jq in range(4):
                            for cg in range(2):
                                for kp in range(WKC // 2):
                                    emit_bank8(jq, cg, kp)
                    else:
                        for kp in range(WKC // 2):
                            for jq in range(4):
                                for cg in range(2):
                                    emit_bank8(jq, cg, kp)
                else:
                    def emit_bank(jq, cg, s):
                        k = kc * WKC + s
                        nc.tensor.matmul(
                            ps_cg[jq * 2 + cg][:, :],
                            lhsT=wv[:, s, 0, jq * 128:(jq + 1) * 128],
                            rhs=xh_sb[:, k * NW + cg * N:k * NW + (cg + 1) * N],
                            start=(k == 0),
                            stop=(k == KH - 1),
                        )

                    if (kc == 0 and q > 0) or kc == NKC - 1:
                        for jq in range(4):
                            for cg in range(2):
                                for s in range(WKC):
                                    emit_bank(jq, cg, s)
                    else:
                        for s in range(WKC):
                            for jq in range(4):
                                for cg in range(2):
                                    emit_bank(jq, cg, s)
            # drain quad q in next-use order; bias (+ scale) fused.
            # one stage per j-tile (SBUF is tight in f32 mode)
            for jq in range(4):
                stage = st_p.tile([128, 2 * N], f32,
                                  name=f"stage_{q}_{jq}", tag="stage")
                j = 4 * q + jq
                for cg in range(2):
                    i = jq * 2 + cg
                    dst = stage[:, cg * N:(cg + 1) * N]
                    # all drains on the Activation queue: DVE runs the scans,
                    # and the W stream lives on SP
                    nc.scalar.activation(
                        dst, ps_cg[i][:, :], Act.Identity,
                        bias=b1_sb[:, j:j + 1], scale=DSC)
                # stage (cg, x) -> in_b rows (cg, jq fixed, partition)
                dst = in_b[q][:, :].rearrange(
                    "(cg jq p) x -> p cg jq x", p=128, cg=2)[:, :, jq:jq + 1, :]
                nc.gpsimd.dma_start(
                    dst,
                    stage[:, :].rearrange("p (cg o x) -> p cg o x", cg=2, o=1),
                )
            nc.gpsimd.collective_compute(
                "ReduceScatter", Alu.add,
                replica_groups=[[0, 1], [2, 3], [4, 5], [6, 7]],
                ins=[in_b[q].opt()], outs=[out_b[q].opt()],
            )
            nc.gpsimd.dma_start(
                h_all[:, 4 * q * N:(4 * q + 4) * N].rearrange(
                    "p (jq x) -> p jq x", jq=4),
                out_b[q][:, :].rearrange("(jq p) x -> p jq x", p=128),
            )
            # hidden LIF scan for this quad (DVE; overlaps later PE quads)
            if q < 3:
                for g in range(NG):
                    scan_group(g, 4 * q, 4 * q + 4)

        # ---- tail: last quad scan + omm + memo, pipelined in T-groups ----
        pos = {}
        for g in range(NG):
            scan_group(g, 12, 16)
            if g >= 1:
                memo_group(g - 1, pos[g - 1])
            pos[g] = omm_group(g)
        memo_group(NG - 1, pos[NG - 1])

        res = sm_p.tile([O, BL], f32)
        nc.vector.tensor_reduce(
            res[:, :],
            so_all[:, :].rearrange("p (t b) -> p b t", t=T),
            axis=mybir.AxisListType.X,
            op=Alu.add,
        )
        nc.sync.dma_start(out_d, res[:, :])

    _legalize_waits(nc, mybir)
    return nc


def _q8(a):
    import ml_dtypes
    return a.astype(ml_dtypes.float8_e4m3fn).astype(np.float32)


def _prep_inputs(x, W1, b1, Wo, bo, mode):
    import ml_dtypes
    f8 = ml_dtypes.float8_e4m3fn

    fp8 = mode == "fp8dr"
    x = np.ascontiguousarray(x, dtype=np.float32)
    xf = x.reshape(B, T, F)
    b1c = np.ascontiguousarray(b1.astype(np.float32).reshape(JT, 128).T)
    b1z = np.zeros_like(b1c)
    wot = np.ascontiguousarray(
        Wo.astype(np.float32).reshape(O, JT, 128).transpose(2, 1, 0).reshape(128, JT * O)
    )
    bo2 = np.ascontiguousarray(bo.astype(np.float32).reshape(O, 1))
    FH = F // 2

    wS = (W1.T.astype(np.float32) * np.float32(SW)).astype(np.float32)  # [F, HID]
    if fp8:
        wh_b = wS.astype(f8)
        wh = wh_b.astype(np.float32)
        w2_b = (_q8((wS - wh) * np.float32(16.0)) / np.float32(16.0)).astype(f8)
        w3_b = (wh / np.float32(4.0)).astype(f8)
        xS = (xf * np.float32(SX)).astype(np.float32)
        xh_b = xS.astype(f8)
        xh = xh_b.astype(np.float32)
        x2_b = (_q8((xS - xh) * np.float32(16.0)) / np.float32(4.0)).astype(f8)
        # per-core [F, N] transposed slices, as raw fp8
        xh_t = [np.ascontiguousarray(
            xh_b.reshape(B, T, F)[c * BL:(c + 1) * BL].transpose(2, 1, 0).reshape(F, N))
            for c in range(NCORES)]
        x2_t = [np.ascontiguousarray(
            x2_b.reshape(B, T, F)[c * BL:(c + 1) * BL].transpose(2, 1, 0).reshape(F, N))
            for c in range(NCORES)]
        # region-interleaved W rows: [KT, 3, 128, HID] -> [(k r p), h]
        w_all = np.stack([
            wh_b.reshape(KT, 128, HID),
            w2_b.reshape(KT, 128, HID),
            w3_b.reshape(KT, 128, HID),
        ], axis=1).reshape(KT * 3 * 128, HID)
    else:
        xts = [np.ascontiguousarray(
            xf[c * BL:(c + 1) * BL].transpose(2, 1, 0).reshape(F, N))
            for c in range(NCORES)]
        w1t = np.ascontiguousarray(W1.T, dtype=np.float32)

    in_maps = []
    for c in range(NCORES):
        lo = c & ~1
        half = c & 1
        kr = slice(half * FH, (half + 1) * FH)
        m = {
            "b1c": (b1c if half == 0 else b1z),
            "wot": wot, "bo2": bo2,
        }
        if fp8:
            krr = slice(half * (KH * 3 * 128), (half + 1) * (KH * 3 * 128))
            m["xh8"] = np.ascontiguousarray(
                np.concatenate([xh_t[lo][kr], xh_t[lo + 1][kr]], axis=1))
            m["xl8"] = np.ascontiguousarray(
                np.concatenate([x2_t[lo][kr], x2_t[lo + 1][kr]], axis=1))
            m["w8"] = np.ascontiguousarray(w_all[krr])
        else:
            m["xt2b"] = np.ascontiguousarray(
                np.concatenate([xts[lo][kr], xts[lo + 1][kr]], axis=1))
            m["w1th"] = np.ascontiguousarray(w1t[kr])
        in_maps.append(m)
    return in_maps


def kernel(x, W1, b1, Wo, bo):
    from concourse import bass_utils

    mode = "fp8dr" if MM_MODE == "fp8dr" else "f32r"
    if "nc" not in _cache:
        _cache["nc"] = _build(mode)
    nc = _cache["nc"]

    in_maps = _prep_inputs(x, W1, b1, Wo, bo, mode)
    trace = os.environ.get("KERNEL_TRACE", "0") == "1"
    last_exc = None
    for _attempt in range(3):
        try:
            res = bass_utils.run_bass_kernel_spmd(
                nc, in_maps, core_ids=list(range(NCORES)), trace=trace
            )
            break
        except Exception as e:
            last_exc = e
    else:
        raise last_exc
    if trace and res.exec_time_ns is not None:
        print(f"HW exec time: {res.exec_time_ns} ns")
        _cache["exec_time_ns"] = res.exec_time_ns

    out = np.empty((B, O), dtype=np.float32)
    for c in range(NCORES):
        out[c * BL:(c + 1) * BL, :] = res.results[c]["out"].T
    return out


# revision 9
# speedup vs baseline: 1.0177x; 1.0177x over previous
"""Trainium2 Bass kernel for nn_BClassifier (spiking MLP classifier), v2/v3.

Data-parallel over batch: 128 samples -> 16 per NeuronCore (8 cores).
HBM-stack partner cores (2c, 2c+1) split the F=12288 contraction in half
(pair k-split); each core computes partial h for BOTH batches of the pair
and a per-quad (4 hidden tiles) ReduceScatter(add) completes h.

v2 (MM_MODE=f32r): fc1 in float32r, bit-exact h.
v3 (MM_MODE=fp8dr): fc1 via fp8e4m3 DoubleRow matmuls (0.5 cyc/row, 2
  k-planes per instruction). Three-term decomposition at native scale:
    h*SX*SW = xh@wh + xh@w2 + x2@w3
  with xh=q8(x*SX), x2=q8(q8((x*SX-xh)*16)/4) ~= 4*(x*SX-xh),
  wh=q8(W.T*SW), w2=q8(q8((W.T*SW-wh)*16)/16) ~= W.T*SW-wh, w3=q8(wh/4).
  All three terms accumulate into one PSUM; the drain applies 1/(SX*SW).

Structure for engine overlap:
  - 4 j-quads; per quad: k-outer matmul loop into 8 PSUM banks, drain with
    bias (even core) in next-use order, DMA to DRAM bounce, per-quad
    ReduceScatter overlapped with the next quad's matmuls, readback, then
    the DVE hidden-LIF scan for that quad.
  - The output-layer matmul is split by j-quad and emitted one quad late so
    the PE queue never waits on DVE. The tiny memo scan runs at the end,
    pipelined with the last quad's hidden scan in T-groups.

Infrastructure note: this walrus build accepts only ONE sync wait per
instruction; _legalize_waits splits Tile's multi-waits onto NoOps.
"""

import os
import sys

import numpy as np

sys.path.insert(0, "/opt/trn_rl_repo")

B, T, C, HH, WW = 128, 25, 3, 64, 64
F = C * HH * WW            # 12288
HID, O = 2048, 2
NCORES = 8
BL = B // NCORES           # 16 samples per core
N = T * BL                 # 400 matmul moving columns per batch group
KT = F // 128              # 96 contraction tiles
KH = KT // 2               # 48 k-tiles per core (pair k-split)
JT = HID // 128            # 16 hidden tiles
NW = 2 * N                 # both batches' columns
BETA = 0.9
THR = 1.0
TG = 5                     # timesteps per scan/omm/memo group
NG = T // TG

MM_MODE = os.environ.get("MM_MODE", "fp8dr")
SW = float(os.environ.get("FP8_SW", "128"))
SX = float(os.environ.get("FP8_SX", "1.41"))

_cache = {}


def _legalize_waits(nc, mybir):
    """Split multi-waits onto standalone NoOps (single wait slot per inst)."""
    import bass_rust

    n = 0
    for f in nc.m.functions:
        new_blocks = []
        changed = False
        for bb in f.blocks:
            out = []
            for inst in bb.instructions:
                si = inst.sync_info
                if si and len(si.on_wait) > 1:
                    changed = True
                    waits = list(si.on_wait)
                    for w in waits[:-1]:
                        n += 1
                        out.append(mybir.InstNoOp(
                            name=f"WSPLIT-{n}",
                            engine=inst.engine,
                            ins=[], outs=[],
                            sync_info=mybir.SyncInfo(on_wait=[w], on_update=[]),
                        ))
                    inst.sync_info = mybir.SyncInfo(
                        on_wait=[waits[-1]], on_update=list(si.on_update))
                out.append(inst)
            new_blocks.append(bass_rust.BasicBlock(
                name=bb.name, instructions=out,
                IsPredicated=bb.IsPredicated, IsExit=bb.IsExit,
                IsLoopEntry=bb.IsLoopEntry,
            ))
        if changed:
            f.blocks = new_blocks


def _build(mode):
    import concourse.bass as bass
    import concourse.tile as tile
    from concourse import mybir
    from contextlib import ExitStack

    f32 = mybir.dt.float32
    Alu = mybir.AluOpType
    Act = mybir.ActivationFunctionType

    fp8 = mode == "fp8dr"
    if fp8:
        mm_dt = mybir.dt.float8e4
        NR = 3                 # weight/x regions per k-tile
        WKC = 4                # k-tiles per W chunk (even: DoubleRow pairs)
        DSC = 1.0 / (SX * SW)  # drain scale
    else:
        mm_dt = {"f32": f32, "f32r": mybir.dt.float32r}[MM_MODE]
        NR = 1
        WKC = 3
        DSC = 1.0
    NKC = KH // WKC            # W chunks per quad

    nc = bass.Bass("TRN2", target_bir_lowering=False, debug=False,
                   num_devices=NCORES)
    if fp8:
        xh_d = nc.dram_tensor("xh8", [KH * 128, NW], mm_dt, kind="ExternalInput").ap()
        xl_d = nc.dram_tensor("xl8", [KH * 128, NW], mm_dt, kind="ExternalInput").ap()
        w_d = nc.dram_tensor("w8", [KH * NR * 128, HID], mm_dt, kind="ExternalInput").ap()
    else:
        xh_d = nc.dram_tensor("xt2b", [KH * 128, NW], mm_dt, kind="ExternalInput").ap()
        xl_d = None
        w_d = nc.dram_tensor("w1th", [KH * 128, HID], mm_dt, kind="ExternalInput").ap()
    b1_d = nc.dram_tensor("b1c", [128, JT], f32, kind="ExternalInput").ap()
    wot_d = nc.dram_tensor("wot", [128, O * JT], f32, kind="ExternalInput").ap()
    bo_d = nc.dram_tensor("bo2", [O, 1], f32, kind="ExternalInput").ap()
    out_d = nc.dram_tensor("out", [O, BL], f32, kind="ExternalOutput").ap()

    with tile.TileContext(nc) as tc, ExitStack() as ctx:
        const_p = ctx.enter_context(tc.tile_pool(name="const", bufs=1))
        xt_p = ctx.enter_context(tc.tile_pool(name="xt", bufs=1))
        w_p = ctx.enter_context(tc.tile_pool(name="w", bufs=4 if fp8 else 3))
        h_p = ctx.enter_context(tc.tile_pool(name="h", bufs=1))
        st_p = ctx.enter_context(tc.tile_pool(name="st", bufs=2))
        ps_p = ctx.enter_context(tc.tile_pool(name="ps", bufs=8, space="PSUM"))

        sm_p = ctx.enter_context(tc.tile_pool(name="sm", bufs=1))
        dram_p = ctx.enter_context(tc.tile_pool(name="dram", bufs=1, space="DRAM"))

        b1_sb = const_p.tile([128, JT], f32)
        wot_sb = const_p.tile([128, O * JT], f32)
        bo_sb = const_p.tile([O, 1], f32)

        def load_consts():
            nc.scalar.dma_start(b1_sb[:, :], b1_d)
            nc.scalar.dma_start(wot_sb[:, :], wot_d)
            nc.scalar.dma_start(bo_sb[:, :], bo_d)

        # x resident in SBUF: per region [128, KH*NW], col = k*NW + cg*N + tb
        xh_sb = xt_p.tile([128, KH * NW], mm_dt)
        xh_r = xh_d.rearrange("(k p) n -> p k n", p=128)
        if fp8:
            xl_sb = xt_p.tile([128, KH * NW], mm_dt)
            xl_r = xl_d.rearrange("(k p) n -> p k n", p=128)

        XCH = 2    # x chunk granularity (k-tiles): small, to not delay W

        def load_x_chunk(ck):
            k0, k1 = ck * XCH, min((ck + 1) * XCH, KH)
            nc.sync.dma_start(
                xh_sb[:, k0 * NW:k1 * NW].rearrange("p (k n) -> p k n", n=NW),
                xh_r[:, k0:k1, :])
            if fp8:
                nc.sync.dma_start(
                    xl_sb[:, k0 * NW:k1 * NW].rearrange("p (k n) -> p k n", n=NW),
                    xl_r[:, k0:k1, :])

        NXC = (KH + XCH - 1) // XCH

        # final h (then spikes in place): [128, 6400], col = j*400 + t*16 + b
        h_all = h_p.tile([128, JT * T * BL], f32)

        # W DRAM view: [(k r p) h] -> [p, k, r, h-slice]
        w_r = w_d.rearrange("(k r p) h -> p k r h", r=NR, p=128)

        # per-quad bounce buffers for the ReduceScatter
        in_b = [dram_p.tile([2 * 4 * 128, N], f32, name=f"in_b{q}")
                for q in range(4)]
        out_b = [dram_p.tile([4 * 128, N], f32, name=f"out_b{q}")
                 for q in range(4)]

        # ---- phase 2-4 helpers -------------------------------------------
        mem1 = sm_p.tile([128, JT * BL], f32)
        o_sb = sm_p.tile([O, N], f32)
        memo = sm_p.tile([O, BL], f32)
        so_all = sm_p.tile([O, N], f32)
        h4 = h_all[:, :].rearrange("p (g t b) -> p g t b", g=JT, t=T)
        ot = lambda t: o_sb[:, t * BL:(t + 1) * BL]
        st = lambda t: so_all[:, t * BL:(t + 1) * BL]
        def scan_group(g, j0, j1):
            m = mem1[:, j0 * BL:j1 * BL]
            ht = lambda t: h4[:, j0:j1, t, :]
            for t in range(TG * g, TG * (g + 1)):
                if t == 0:
                    nc.vector.tensor_copy(m, ht(0))
                else:
                    nc.vector.scalar_tensor_tensor(
                        m, m, BETA, ht(t), Alu.mult, Alu.add)
                    nc.vector.tensor_tensor(m, m, ht(t - 1), Alu.subtract)
                nc.vector.tensor_scalar(ht(t), m, THR, None, Alu.is_gt)

        def omm_group(g):
            # output-layer matmul for T-group g over all 16 j-tiles
            pog = ps_p.tile([O, TG * BL], f32, name=f"po_{g}", tag="pscg")
            for j in range(JT):
                nc.tensor.matmul(
                    pog[:, :],
                    lhsT=wot_sb[:, O * j:O * (j + 1)],
                    rhs=h_all[:, j * N + g * TG * BL:j * N + (g + 1) * TG * BL],
                    start=(j == 0),
                    stop=(j == JT - 1),
                )
            return pog

        def memo_group(g, pog):
            nc.vector.tensor_scalar(
                o_sb[:, g * TG * BL:(g + 1) * TG * BL],
                pog[:, :], bo_sb[:, 0:1], None, Alu.add)
            for t in range(TG * g, TG * (g + 1)):
                if t == 0:
                    nc.vector.tensor_copy(memo[:, :], ot(0))
                else:
                    nc.vector.scalar_tensor_tensor(
                        memo[:, :], memo[:, :], BETA, ot(t), Alu.mult, Alu.add)
                    nc.vector.tensor_tensor(
                        memo[:, :], memo[:, :], st(t - 1), Alu.subtract)
                nc.vector.tensor_scalar(st(t), memo[:, :], THR, None, Alu.is_gt)

        # ---- phase 1: 4 j-quads of fc1 -----------------------------------
        # Global W-chunk pipeline: prefetch PF chunks ahead (crossing quad
        # boundaries so the bounce DMAs never sit in front of the W stream),
        # with x pieces interleaved just ahead of their first use.
        PF = 3 if fp8 else 2
        chunks = [(q, kc) for q in range(4) for kc in range(NKC)]
        w_tiles = {}
        x_emitted = 0
        consts_loaded = [False]

        def emit_chunk_dma(ci):
            nonlocal x_emitted
            q, kc = chunks[ci]
            # x pieces stay just ahead of the matmul k-position
            if q == 0:
                want = min(NXC, ((kc + 1) * WKC + XCH - 1) // XCH)
                while x_emitted < want:
                    load_x_chunk(x_emitted)
                    x_emitted += 1
            wt = w_p.tile([128, WKC * NR * 512], mm_dt)
            dma_eng = nc.sync
            dma_eng.dma_start(
                wt[:, :].rearrange("p (k r c) -> p k r c", k=WKC, r=NR),
                w_r[:, kc * WKC:(kc + 1) * WKC, :, q * 512:(q + 1) * 512],
            )
            w_tiles[ci] = wt
            if not consts_loaded[0]:
                consts_loaded[0] = True
                load_consts()

        for ci in range(PF):
            emit_chunk_dma(ci)

        for q in range(4):
            ps_cg = [ps_p.tile([128, N], f32, name=f"ps_{q}_{i}", tag="pscg")
                     for i in range(8)]  # index jq*2+cg, issue order
            for kc in range(NKC):
                ci = q * NKC + kc
                if ci + PF < len(chunks):
                    emit_chunk_dma(ci + PF)
                wt = w_tiles.pop(ci)
                wv = wt[:, :].rearrange("p (k r c) -> p k r c", k=WKC, r=NR)
                # first chunk of a quad runs bank-major so the previous
                # quad's PSUM drains are awaited incrementally, not all at
                # once; later chunks run k-major (order within a chunk is
                # free — accumulation is per-bank).
                if fp8:
                    xh_v = xh_sb[:, :].rearrange("p (k c n) -> p k c n", k=KH, c=2)
                    xl_v = xl_sb[:, :].rearrange("p (k c n) -> p k c n", k=KH, c=2)

                    def emit_bank8(jq, cg, kp):
                        k = kc * WKC + 2 * kp
                        kl = 2 * kp
                        pt = ps_cg[jq * 2 + cg]
                        for r, xv in ((0, xh_v), (1, xh_v), (2, xl_v)):
                            nc.tensor.matmul(
                                pt[:, :],
                                lhsT=wv[:, kl:kl + 2, r, jq * 128:(jq + 1) * 128],
                                rhs=xv[:, k:k + 2, cg, :],
                                start=(k == 0 and r == 0),
                                stop=(k == KH - 2 and r == 2),
                                perf_mode=mybir.MatmulPerfMode.DoubleRow,
                            )

                    if (kc == 0 and q > 0) or kc == NKC - 1:
                        for jq in range(4):
                            for cg in range(2):
                                for kp in range(WKC // 2):
                                    emit_bank8(jq, cg, kp)
                    else:
                        for kp in range(WKC // 2):
                            for jq in range(4):
                                for cg in range(2):
                                    emit_bank8(jq, cg, kp)
                else:
                    def emit_bank(jq, cg, s):
                        k = kc * WKC + s
                        nc.tensor.matmul(
                            ps_cg[jq * 2 + cg][:, :],
                            lhsT=wv[:, s, 0, jq * 128:(jq + 1) * 128],
                            rhs=xh_sb[:, k * NW + cg * N:k * NW + (cg + 1) * N],
                            start=(k == 0),
                            stop=(k == KH - 1),
                        )

                    if (kc == 0 and q > 0) or kc == NKC - 1:
                        for jq in range(4):
                            for cg in range(2):
                                for s in range(WKC):
                                    emit_bank(jq, cg, s)
                    else:
                        for s in range(WKC):
                            for jq in range(4):
                                for cg in range(2):
                                    emit_bank(jq, cg, s)
            # drain quad q in next-use order; bias (+ scale) fused.
            # one stage per j-tile (SBUF is tight in f32 mode)
            for jq in range(4):
                stage = st_p.tile([128, 2 * N], f32,
                                  name=f"stage_{q}_{jq}", tag="stage")
                j = 4 * q + jq
                for cg in range(2):
                    i = jq * 2 + cg
                    dst = stage[:, cg * N:(cg + 1) * N]
                    # all drains on the Activation queue: DVE runs the scans,
                    # and the W stream lives on SP
                    nc.scalar.activation(
                        dst, ps_cg[i][:, :], Act.Identity,
                        bias=b1_sb[:, j:j + 1], scale=DSC)
                # stage (cg, x) -> in_b rows (cg, jq fixed, partition)
                dst = in_b[q][:, :].rearrange(
                    "(cg jq p) x -> p cg jq x", p=128, cg=2)[:, :, jq:jq + 1, :]
                nc.gpsimd.dma_start(
                    dst,
                    stage[:, :].rearrange("p (cg o x) -> p cg o x", cg=2, o=1),
                )
            nc.gpsimd.collective_compute(
                "ReduceScatter", Alu.add,
                replica_groups=[[0, 1], [2, 3], [4, 5], [6, 7]],
                ins=[in_b[q].opt()], outs=[out_b[q].opt()],
            )
            nc.gpsimd.dma_start(
                h_all[:, 4 * q * N:(4 * q + 4) * N].rearrange(
                    "p (jq x) -> p jq x", jq=4),
                out_b[q][:, :].rearrange("(jq p) x -> p jq x", p=128),
            )
            # hidden LIF scan for this quad (DVE; overlaps later PE quads)
            if q < 3:
                for g in range(NG):
                    scan_group(g, 4 * q, 4 * q + 4)

        # ---- tail: last quad scan + omm + memo, pipelined in T-groups ----
        pos = {}
        for g in range(NG):
            scan_group(g, 12, 16)
            if g >= 1:
                memo_group(g - 1, pos[g - 1])
            pos[g] = omm_group(g)
        memo_group(NG - 1, pos[NG - 1])

        res = sm_p.tile([O, BL], f32)
        nc.vector.tensor_reduce(
            res[:, :],
            so_all[:, :].rearrange("p (t b) -> p b t", t=T),
            axis=mybir.AxisListType.X,
            op=Alu.add,
        )
        nc.sync.dma_start(out_d, res[:, :])

    _legalize_waits(nc, mybir)
    return nc


def _q8(a):
    import ml_dtypes
    return a.astype(ml_dtypes.float8_e4m3fn).astype(np.float32)


def _prep_inputs(x, W1, b1, Wo, bo, mode):
    import ml_dtypes
    f8 = ml_dtypes.float8_e4m3fn

    fp8 = mode == "fp8dr"
    x = np.ascontiguousarray(x, dtype=np.float32)
    xf = x.reshape(B, T, F)
    b1c = np.ascontiguousarray(b1.astype(np.float32).reshape(JT, 128).T)
    b1z = np.zeros_like(b1c)
    wot = np.ascontiguousarray(
        Wo.astype(np.float32).reshape(O, JT, 128).transpose(2, 1, 0).reshape(128, JT * O)
    )
    bo2 = np.ascontiguousarray(bo.astype(np.float32).reshape(O, 1))
    FH = F // 2

    wS = (W1.T.astype(np.float32) * np.float32(SW)).astype(np.float32)  # [F, HID]
    if fp8:
        wh_b = wS.astype(f8)
        wh = wh_b.astype(np.float32)
        w2_b = (_q8((wS - wh) * np.float32(16.0)) / np.float32(16.0)).astype(f8)
        w3_b = (wh / np.float32(4.0)).astype(f8)
        xS = (xf * np.float32(SX)).astype(np.float32)
        xh_b = xS.astype(f8)
        xh = xh_b.astype(np.float32)
        x2_b = (_q8((xS - xh) * np.float32(16.0)) / np.float32(4.0)).astype(f8)
        # per-core [F, N] transposed slices, as raw fp8
        xh_t = [np.ascontiguousarray(
            xh_b.reshape(B, T, F)[c * BL:(c + 1) * BL].transpose(2, 1, 0).reshape(F, N))
            for c in range(NCORES)]
        x2_t = [np.ascontiguousarray(
            x2_b.reshape(B, T, F)[c * BL:(c + 1) * BL].transpose(2, 1, 0).reshape(F, N))
            for c in range(NCORES)]
        # region-interleaved W rows: [KT, 3, 128, HID] -> [(k r p), h]
        w_all = np.stack([
            wh_b.reshape(KT, 128, HID),
            w2_b.reshape(KT, 128, HID),
            w3_b.reshape(KT, 128, HID),
        ], axis=1).reshape(KT * 3 * 128, HID)
    else:
        xts = [np.ascontiguousarray(
            xf[c * BL:(c + 1) * BL].transpose(2, 1, 0).reshape(F, N))
            for c in range(NCORES)]
        w1t = np.ascontiguousarray(W1.T, dtype=np.float32)

    in_maps = []
    for c in range(NCORES):
        lo = c & ~1
        half = c & 1
        kr = slice(half * FH, (half + 1) * FH)
        m = {
            "b1c": (b1c if half == 0 else b1z),
            "wot": wot, "bo2": bo2,
        }
        if fp8:
            krr = slice(half * (KH * 3 * 128), (half + 1) * (KH * 3 * 128))
            m["xh8"] = np.ascontiguousarray(
                np.concatenate([xh_t[lo][kr], xh_t[lo + 1][kr]], axis=1))
            m["xl8"] = np.ascontiguousarray(
                np.concatenate([x2_t[lo][kr], x2_t[lo + 1][kr]], axis=1))
            m["w8"] = np.ascontiguousarray(w_all[krr])
        else:
            m["xt2b"] = np.ascontiguousarray(
                np.concatenate([xts[lo][kr], xts[lo + 1][kr]], axis=1))
            m["w1th"] = np.ascontiguousarray(w1t[kr])
        in_maps.append(m)
    return in_maps


def kernel(x, W1, b1, Wo, bo):
    from concourse import bass_utils

    mode = "fp8dr" if MM_MODE == "fp8dr" else "f32r"
    if "nc" not in _cache:
        _cache["nc"] = _build(mode)
    nc = _cache["nc"]

    in_maps = _prep_inputs(x, W1, b1, Wo, bo, mode)
    trace = os.environ.get("KERNEL_TRACE", "0") == "1"
    last_exc = None
    for _attempt in range(4):
        try:
            res = bass_utils.run_bass_kernel_spmd(
                nc, in_maps, core_ids=list(range(NCORES)), trace=trace
            )
            break
        except Exception as e:
            last_exc = e
            # transient device wedges (NRT_EXEC_UNIT_UNRECOVERABLE) usually
            # recover with a core reset + backoff
            os.environ["NEURON_RT_RESET_CORES"] = "1"
            import time
            time.sleep(5 * (_attempt + 1))
    else:
        raise last_exc
    if trace and res.exec_time_ns is not None:
        print(f"HW exec time: {res.exec_time_ns} ns")
        _cache["exec_time_ns"] = res.exec_time_ns

    out = np.empty((B, O), dtype=np.float32)
    for c in range(NCORES):
        out[c * BL:(c + 1) * BL, :] = res.results[c]["out"].T
    return out


# revision 11
# speedup vs baseline: 1.0875x; 1.0685x over previous
"""Trainium2 Bass kernel for nn_BClassifier (spiking MLP classifier), v2/v3.

Data-parallel over batch: 128 samples -> 16 per NeuronCore (8 cores).
HBM-stack partner cores (2c, 2c+1) split the F=12288 contraction in half
(pair k-split); each core computes partial h for BOTH batches of the pair
and a per-quad (4 hidden tiles) ReduceScatter(add) completes h.

v2 (MM_MODE=f32r): fc1 in float32r, bit-exact h.
v3 (MM_MODE=fp8dr): fc1 via fp8e4m3 DoubleRow matmuls (0.5 cyc/row, 2
  k-planes per instruction). Three-term decomposition at native scale:
    h*SX*SW = xh@wh + xh@w2 + x2@w3
  with xh=q8(x*SX), x2=q8(q8((x*SX-xh)*16)/4) ~= 4*(x*SX-xh),
  wh=q8(W.T*SW), w2=q8(q8((W.T*SW-wh)*16)/16) ~= W.T*SW-wh, w3=q8(wh/4).
  All three terms accumulate into one PSUM; the drain applies 1/(SX*SW).

Structure for engine overlap:
  - 4 j-quads; per quad: k-outer matmul loop into 8 PSUM banks, drain with
    bias (even core) in next-use order, DMA to DRAM bounce, per-quad
    ReduceScatter overlapped with the next quad's matmuls, readback, then
    the DVE hidden-LIF scan for that quad.
  - The output-layer matmul is split by j-quad and emitted one quad late so
    the PE queue never waits on DVE. The tiny memo scan runs at the end,
    pipelined with the last quad's hidden scan in T-groups.

Infrastructure note: this walrus build accepts only ONE sync wait per
instruction; _legalize_waits splits Tile's multi-waits onto NoOps.
"""

import os
import sys

import numpy as np

sys.path.insert(0, "/opt/trn_rl_repo")

B, T, C, HH, WW = 128, 25, 3, 64, 64
F = C * HH * WW            # 12288
HID, O = 2048, 2
NCORES = 8
BL = B // NCORES           # 16 samples per core
N = T * BL                 # 400 matmul moving columns per batch group
KT = F // 128              # 96 contraction tiles
KH = KT // 2               # 48 k-tiles per core (pair k-split)
JT = HID // 128            # 16 hidden tiles
NW = 2 * N                 # both batches' columns
BETA = 0.9
THR = 1.0
TG = 5                     # timesteps per scan/omm/memo group
NG = T // TG

MM_MODE = os.environ.get("MM_MODE", "fp8dr")
SW = float(os.environ.get("FP8_SW", "128"))
SX = float(os.environ.get("FP8_SX", "1.41"))

_cache = {}


def _legalize_waits(nc, mybir):
    """Split multi-waits onto standalone NoOps (single wait slot per inst)."""
    import bass_rust

    n = 0
    for f in nc.m.functions:
        new_blocks = []
        changed = False
        for bb in f.blocks:
            out = []
            for inst in bb.instructions:
                si = inst.sync_info
                if si and len(si.on_wait) > 1:
                    changed = True
                    waits = list(si.on_wait)
                    for w in waits[:-1]:
                        n += 1
                        out.append(mybir.InstNoOp(
                            name=f"WSPLIT-{n}",
                            engine=inst.engine,
                            ins=[], outs=[],
                            sync_info=mybir.SyncInfo(on_wait=[w], on_update=[]),
                        ))
                    inst.sync_info = mybir.SyncInfo(
                        on_wait=[waits[-1]], on_update=list(si.on_update))
                out.append(inst)
            new_blocks.append(bass_rust.BasicBlock(
                name=bb.name, instructions=out,
                IsPredicated=bb.IsPredicated, IsExit=bb.IsExit,
                IsLoopEntry=bb.IsLoopEntry,
            ))
        if changed:
            f.blocks = new_blocks


def _build(mode):
    import concourse.bass as bass
    import concourse.tile as tile
    from concourse import mybir
    from contextlib import ExitStack

    f32 = mybir.dt.float32
    Alu = mybir.AluOpType
    Act = mybir.ActivationFunctionType

    fp8 = mode == "fp8dr"
    if fp8:
        mm_dt = mybir.dt.float8e4
        NR = 3                 # weight/x regions per k-tile
        WKC = 4                # k-tiles per W chunk (even: DoubleRow pairs)
        DSC = 1.0 / (SX * SW)  # drain scale
    else:
        mm_dt = {"f32": f32, "f32r": mybir.dt.float32r}[MM_MODE]
        NR = 1
        WKC = 3
        DSC = 1.0
    NKC = KH // WKC            # W chunks per quad

    nc = bass.Bass("TRN2", target_bir_lowering=False, debug=False,
                   num_devices=NCORES)
    if fp8:
        xh_d = nc.dram_tensor("xh8", [KH * 128, NW], mm_dt, kind="ExternalInput").ap()
        xl_d = nc.dram_tensor("xl8", [KH * 128, NW], mm_dt, kind="ExternalInput").ap()
        w_d = nc.dram_tensor("w8", [KH * NR * 128, HID], mm_dt, kind="ExternalInput").ap()
        # last quad (j12-15) runs the FULL F contraction per core for its OWN
        # batch (no ReduceScatter): extra W (other k-half, j12-15 cols) and a
        # streamed own-batch x copy, packed 2 k-tiles per row (800B descs)
        xq4h_d = nc.dram_tensor("xq4h", [(KT // 2) * 128, NW], mm_dt, kind="ExternalInput").ap()
        xq4l_d = nc.dram_tensor("xq4l", [(KT // 2) * 128, NW], mm_dt, kind="ExternalInput").ap()
        wq4_d = nc.dram_tensor("wq4", [KH * NR * 128, 512], mm_dt, kind="ExternalInput").ap()
        b1f_d = nc.dram_tensor("b1f", [128, JT], f32, kind="ExternalInput").ap()
    else:
        xh_d = nc.dram_tensor("xt2b", [KH * 128, NW], mm_dt, kind="ExternalInput").ap()
        xl_d = None
        w_d = nc.dram_tensor("w1th", [KH * 128, HID], mm_dt, kind="ExternalInput").ap()
    b1_d = nc.dram_tensor("b1c", [128, JT], f32, kind="ExternalInput").ap()
    wot_d = nc.dram_tensor("wot", [128, O * JT], f32, kind="ExternalInput").ap()
    bo_d = nc.dram_tensor("bo2", [O, 1], f32, kind="ExternalInput").ap()
    out_d = nc.dram_tensor("out", [O, BL], f32, kind="ExternalOutput").ap()

    with tile.TileContext(nc) as tc, ExitStack() as ctx:
        const_p = ctx.enter_context(tc.tile_pool(name="const", bufs=1))
        xt_p = ctx.enter_context(tc.tile_pool(name="xt", bufs=1))
        w_p = ctx.enter_context(tc.tile_pool(name="w", bufs=int(os.environ.get("WBUFS", "6")) if fp8 else 3))
        h_p = ctx.enter_context(tc.tile_pool(name="h", bufs=1))
        st_p = ctx.enter_context(tc.tile_pool(name="st", bufs=2))
        ps_p = ctx.enter_context(tc.tile_pool(name="ps", bufs=8, space="PSUM"))

        sm_p = ctx.enter_context(tc.tile_pool(name="sm", bufs=1))
        dram_p = ctx.enter_context(tc.tile_pool(name="dram", bufs=1, space="DRAM"))

        b1_sb = const_p.tile([128, JT], f32)
        wot_sb = const_p.tile([128, O * JT], f32)
        bo_sb = const_p.tile([O, 1], f32)
        if fp8:
            b1f_sb = const_p.tile([128, JT], f32, name="b1f_sb")
        else:
            b1f_sb = b1_sb

        def load_consts():
            nc.scalar.dma_start(b1_sb[:, :], b1_d)
            nc.scalar.dma_start(wot_sb[:, :], wot_d)
            nc.scalar.dma_start(bo_sb[:, :], bo_d)
            if fp8:
                nc.scalar.dma_start(b1f_sb[:, :], b1f_d)

        # x resident in SBUF: per region [128, KH*NW], col = k*NW + cg*N + tb
        xh_sb = xt_p.tile([128, KH * NW], mm_dt)
        xh_r = xh_d.rearrange("(k p) n -> p k n", p=128)
        if fp8:
            xl_sb = xt_p.tile([128, KH * NW], mm_dt)
            xl_r = xl_d.rearrange("(k p) n -> p k n", p=128)

        XCH = 4    # x chunk granularity (k-tiles)

        def load_x_chunk(ck):
            k0, k1 = ck * XCH, min((ck + 1) * XCH, KH)
            nc.sync.dma_start(
                xh_sb[:, k0 * NW:k1 * NW].rearrange("p (k n) -> p k n", n=NW),
                xh_r[:, k0:k1, :])
            if fp8:
                nc.sync.dma_start(
                    xl_sb[:, k0 * NW:k1 * NW].rearrange("p (k n) -> p k n", n=NW),
                    xl_r[:, k0:k1, :])

        NXC = (KH + XCH - 1) // XCH

        # final h (then spikes in place): [128, 6400], col = j*400 + t*16 + b
        h_all = h_p.tile([128, JT * T * BL], f32)

        # W DRAM view: [(k r p) h] -> [p, k, r, h-slice]
        w_r = w_d.rearrange("(k r p) h -> p k r h", r=NR, p=128)
        if fp8:
            wq4_r = wq4_d.rearrange("(k r p) h -> p k r h", r=NR, p=128)
            xq4h_r = xq4h_d.rearrange("(k p) n -> p k n", p=128)
            xq4l_r = xq4l_d.rearrange("(k p) n -> p k n", p=128)
            xq_p = ctx.enter_context(tc.tile_pool(name="xq", bufs=int(os.environ.get("XQBUFS", "8"))))

        NPQ = 3 if fp8 else 4      # number of paired (ReduceScatter) quads
        # per-quad bounce buffers for the ReduceScatter
        in_b = [dram_p.tile([2 * 4 * 128, N], f32, name=f"in_b{q}")
                for q in range(NPQ)]
        out_b = [dram_p.tile([4 * 128, N], f32, name=f"out_b{q}")
                 for q in range(NPQ)]

        # ---- phase 2-4 helpers -------------------------------------------
        mem1 = sm_p.tile([128, JT * BL], f32)
        o_sb = sm_p.tile([O, N], f32)
        memo = sm_p.tile([O, BL], f32)
        so_all = sm_p.tile([O, N], f32)
        h4 = h_all[:, :].rearrange("p (g t b) -> p g t b", g=JT, t=T)
        ot = lambda t: o_sb[:, t * BL:(t + 1) * BL]
        st = lambda t: so_all[:, t * BL:(t + 1) * BL]
        def scan_group(g, j0, j1):
            m = mem1[:, j0 * BL:j1 * BL]
            ht = lambda t: h4[:, j0:j1, t, :]
            for t in range(TG * g, TG * (g + 1)):
                if t == 0:
                    nc.vector.tensor_copy(m, ht(0))
                else:
                    nc.vector.scalar_tensor_tensor(
                        m, m, BETA, ht(t), Alu.mult, Alu.add)
                    nc.vector.tensor_tensor(m, m, ht(t - 1), Alu.subtract)
                nc.vector.tensor_scalar(ht(t), m, THR, None, Alu.is_gt)

        def omm_group(g):
            # output-layer matmul for T-group g over all 16 j-tiles
            pog = ps_p.tile([O, TG * BL], f32, name=f"po_{g}", tag="pscg")
            for j in range(JT):
                nc.tensor.matmul(
                    pog[:, :],
                    lhsT=wot_sb[:, O * j:O * (j + 1)],
                    rhs=h_all[:, j * N + g * TG * BL:j * N + (g + 1) * TG * BL],
                    start=(j == 0),
                    stop=(j == JT - 1),
                )
            return pog

        def memo_group(g, pog):
            nc.vector.tensor_scalar(
                o_sb[:, g * TG * BL:(g + 1) * TG * BL],
                pog[:, :], bo_sb[:, 0:1], None, Alu.add)
            for t in range(TG * g, TG * (g + 1)):
                if t == 0:
                    nc.vector.tensor_copy(memo[:, :], ot(0))
                else:
                    nc.vector.scalar_tensor_tensor(
                        memo[:, :], memo[:, :], BETA, ot(t), Alu.mult, Alu.add)
                    nc.vector.tensor_tensor(
                        memo[:, :], memo[:, :], st(t - 1), Alu.subtract)
                nc.vector.tensor_scalar(st(t), memo[:, :], THR, None, Alu.is_gt)

        # ---- phase 1: 4 j-quads of fc1 -----------------------------------
        # Global W-chunk pipeline: prefetch PF chunks ahead (crossing quad
        # boundaries so the bounce DMAs never sit in front of the W stream),
        # with x pieces interleaved just ahead of their first use.
        PF = 3 if fp8 else 2
        NKC4 = KT // WKC           # chunks in the full-F own-batch quad
        if fp8:
            chunks = ([(q, kc) for q in range(3) for kc in range(NKC)]
                      + [(3, kc) for kc in range(NKC4)])
        else:
            chunks = [(q, kc) for q in range(4) for kc in range(NKC)]
        w_tiles = {}
        xq_tiles = {}
        x_emitted = 0
        consts_loaded = [False]

        def emit_chunk_dma(ci):
            nonlocal x_emitted
            q, kc = chunks[ci]
            wt = w_p.tile([128, WKC * NR * 512], mm_dt)
            if fp8 and q == 3 and os.environ.get("Q4_HIPRI", "0") == "1":
                with tc.high_priority(offset=2000):
                    if kc < NKC:
                        nc.sync.dma_start(
                            wt[:, :].rearrange("p (k r c) -> p k r c", k=WKC, r=NR),
                            w_r[:, kc * WKC:(kc + 1) * WKC, :, 3 * 512:4 * 512],
                        )
                    else:
                        kc2 = kc - NKC
                        nc.sync.dma_start(
                            wt[:, :].rearrange("p (k r c) -> p k r c", k=WKC, r=NR),
                            wq4_r[:, kc2 * WKC:(kc2 + 1) * WKC, :, :],
                        )
                    w_tiles[ci] = wt
                    if kc % 2 == 0:
                        xqt = xq_p.tile([128, 8 * NW], mm_dt)
                        nc.sync.dma_start(
                            xqt[:, 0:4 * NW].rearrange("p (k n) -> p k n", n=NW),
                            xq4h_r[:, kc * 2:kc * 2 + 4, :])
                        nc.sync.dma_start(
                            xqt[:, 4 * NW:8 * NW].rearrange("p (k n) -> p k n", n=NW),
                            xq4l_r[:, kc * 2:kc * 2 + 4, :])
                        xq_tiles[ci] = (xqt, 0)
                        xq_tiles[ci + 1] = (xqt, 2 * NW)
                return
            if fp8 and q == 3:
                if kc < NKC:
                    # own k-half of j12-15 (last column block of w8)
                    nc.sync.dma_start(
                        wt[:, :].rearrange("p (k r c) -> p k r c", k=WKC, r=NR),
                        w_r[:, kc * WKC:(kc + 1) * WKC, :, 3 * 512:4 * 512],
                    )
                else:
                    kc2 = kc - NKC
                    nc.sync.dma_start(
                        wt[:, :].rearrange("p (k r c) -> p k r c", k=WKC, r=NR),
                        wq4_r[:, kc2 * WKC:(kc2 + 1) * WKC, :, :],
                    )
                w_tiles[ci] = wt
                # own-batch x: one load covers TWO W chunks (4 k-pairs),
                # both regions — keeps the instruction/semaphore count low
                if kc % 2 == 0:
                    xqt = xq_p.tile([128, 8 * NW], mm_dt)
                    nc.sync.dma_start(
                        xqt[:, 0:4 * NW].rearrange("p (k n) -> p k n", n=NW),
                        xq4h_r[:, kc * 2:kc * 2 + 4, :])
                    nc.sync.dma_start(
                        xqt[:, 4 * NW:8 * NW].rearrange("p (k n) -> p k n", n=NW),
                        xq4l_r[:, kc * 2:kc * 2 + 4, :])
                    xq_tiles[ci] = (xqt, 0)
                    xq_tiles[ci + 1] = (xqt, 2 * NW)
                return
            # x pieces stay just ahead of the matmul k-position
            if q == 0:
                want = min(NXC, ((kc + 1) * WKC + XCH - 1) // XCH)
                while x_emitted < want:
                    load_x_chunk(x_emitted)
                    x_emitted += 1
            nc.sync.dma_start(
                wt[:, :].rearrange("p (k r c) -> p k r c", k=WKC, r=NR),
                w_r[:, kc * WKC:(kc + 1) * WKC, :, q * 512:(q + 1) * 512],
            )
            w_tiles[ci] = wt
            if not consts_loaded[0]:
                consts_loaded[0] = True
                load_consts()

        for ci in range(PF):
            emit_chunk_dma(ci)

        for q in range(4):
            own_quad = fp8 and q == 3
            nkc_q = NKC4 if own_quad else NKC
            ps_cg = [ps_p.tile([128, N], f32, name=f"ps_{q}_{i}", tag="pscg")
                     for i in range(4 if own_quad else 8)]
            for kc in range(nkc_q):
                ci = q * NKC + kc
                if ci + PF < len(chunks):
                    emit_chunk_dma(ci + PF)
                wt = w_tiles.pop(ci)
                wv = wt[:, :].rearrange("p (k r c) -> p k r c", k=WKC, r=NR)
                # first chunk of a quad runs bank-major so the previous
                # quad's PSUM drains are awaited incrementally, not all at
                # once; later chunks run k-major (order within a chunk is
                # free — accumulation is per-bank).
                if own_quad:
                    xqt, xbase = xq_tiles.pop(ci)

                    def emit_bank4(jq, kp):
                        k = kc * WKC + 2 * kp
                        kl = 2 * kp
                        pt = ps_cg[jq]
                        for r, xoff in ((0, 0), (1, 0), (2, 4 * NW)):
                            off = xbase + xoff + kp * NW
                            nc.tensor.matmul(
                                pt[:, :],
                                lhsT=wv[:, kl:kl + 2, r, jq * 128:(jq + 1) * 128],
                                rhs=xqt[:, off:off + NW]
                                    .rearrange("p (two n) -> p two n", two=2),
                                start=(k == 0 and r == 0),
                                stop=(k == KT - 2 and r == 2),
                                perf_mode=mybir.MatmulPerfMode.DoubleRow,
                            )

                    if kc == 0 or kc == nkc_q - 1:
                        for jq in range(4):
                            for kp in range(WKC // 2):
                                emit_bank4(jq, kp)
                    else:
                        for kp in range(WKC // 2):
                            for jq in range(4):
                                emit_bank4(jq, kp)
                elif fp8:
                    xh_v = xh_sb[:, :].rearrange("p (k c n) -> p k c n", k=KH, c=2)
                    xl_v = xl_sb[:, :].rearrange("p (k c n) -> p k c n", k=KH, c=2)

                    def emit_bank8(jq, cg, kp):
                        k = kc * WKC + 2 * kp
                        kl = 2 * kp
                        pt = ps_cg[jq * 2 + cg]
                        for r, xv in ((0, xh_v), (1, xh_v), (2, xl_v)):
                            nc.tensor.matmul(
                                pt[:, :],
                                lhsT=wv[:, kl:kl + 2, r, jq * 128:(jq + 1) * 128],
                                rhs=xv[:, k:k + 2, cg, :],
                                start=(k == 0 and r == 0),
                                stop=(k == KH - 2 and r == 2),
                                perf_mode=mybir.MatmulPerfMode.DoubleRow,
                            )

                    if (kc == 0 and q > 0) or kc == NKC - 1:
                        for jq in range(4):
                            for cg in range(2):
                                for kp in range(WKC // 2):
                                    emit_bank8(jq, cg, kp)
                    else:
                        for kp in range(WKC // 2):
                            for jq in range(4):
                                for cg in range(2):
                                    emit_bank8(jq, cg, kp)
                else:
                    def emit_bank(jq, cg, s):
                        k = kc * WKC + s
                        nc.tensor.matmul(
                            ps_cg[jq * 2 + cg][:, :],
                            lhsT=wv[:, s, 0, jq * 128:(jq + 1) * 128],
                            rhs=xh_sb[:, k * NW + cg * N:k * NW + (cg + 1) * N],
                            start=(k == 0),
                            stop=(k == KH - 1),
                        )

                    if (kc == 0 and q > 0) or kc == NKC - 1:
                        for jq in range(4):
                            for cg in range(2):
                                for s in range(WKC):
                                    emit_bank(jq, cg, s)
                    else:
                        for s in range(WKC):
                            for jq in range(4):
                                for cg in range(2):
                                    emit_bank(jq, cg, s)
            if q == 2 and fp8:
                # steer the scheduler's fixed-count block-split barrier into
                # this RS-free window (between quad-2 compute and its RS):
                # filler nops shift the split point earlier in program order
                for _ in range(int(os.environ.get("SPLIT_NOPS", "0"))):
                    nc.sync.nop()
            if own_quad:
                # h for j12-15 is complete in PSUM (full F, own batch):
                # drain straight into h_all with the FULL bias — no exchange
                for jq in range(4):
                    j = 12 + jq
                    nc.scalar.activation(
                        h_all[:, j * N:(j + 1) * N], ps_cg[jq][:, :],
                        Act.Identity, bias=b1f_sb[:, j:j + 1], scale=DSC)
                continue
            # drain quad q in next-use order; bias (+ scale) fused.
            # one stage per j-tile (SBUF is tight in f32 mode)
            for jq in range(4):
                stage = st_p.tile([128, 2 * N], f32,
                                  name=f"stage_{q}_{jq}", tag="stage")
                j = 4 * q + jq
                for cg in range(2):
                    i = jq * 2 + cg
                    dst = stage[:, cg * N:(cg + 1) * N]
                    # all drains on the Activation queue: DVE runs the scans,
                    # and the W stream lives on SP
                    nc.scalar.activation(
                        dst, ps_cg[i][:, :], Act.Identity,
                        bias=b1_sb[:, j:j + 1], scale=DSC)
                # stage (cg, x) -> in_b rows (cg, jq fixed, partition)
                dst = in_b[q][:, :].rearrange(
                    "(cg jq p) x -> p cg jq x", p=128, cg=2)[:, :, jq:jq + 1, :]
                _stage_eng = {"pool": nc.gpsimd, "dve": nc.vector,
                              "act": nc.scalar}[os.environ.get("STAGE_Q", "act")]
                _stage_eng.dma_start(
                    dst,
                    stage[:, :].rearrange("p (cg o x) -> p cg o x", cg=2, o=1),
                )
            nc.gpsimd.collective_compute(
                "ReduceScatter", Alu.add,
                replica_groups=[[0, 1], [2, 3], [4, 5], [6, 7]],
                ins=[in_b[q].opt()], outs=[out_b[q].opt()],
            )
            _rb_eng = {"pool": nc.gpsimd, "dve": nc.vector,
                       "act": nc.scalar}[os.environ.get("RB_Q", "act")]
            if fp8 and q == 2:
                # defer the last paired quad's readback+scan to the tail: an
                # emitted readback waiting on RS#2 during the own quad
                # head-of-line blocks shared queues/devices and stalls the PE
                continue
            _rb_eng.dma_start(
                h_all[:, 4 * q * N:(4 * q + 4) * N].rearrange(
                    "p (jq x) -> p jq x", jq=4),
                out_b[q][:, :].rearrange("(jq p) x -> p jq x", p=128),
            )
            # hidden LIF scan for this quad (DVE; overlaps later PE quads)
            if q < 3:
                for g in range(NG):
                    scan_group(g, 4 * q, 4 * q + 4)

        # ---- tail: last quad scan + omm + memo, pipelined in T-groups ----
        if fp8:
            _rb_eng = {"pool": nc.gpsimd, "dve": nc.vector,
                       "act": nc.scalar}[os.environ.get("RB_Q", "act")]
            _rb_eng.dma_start(
                h_all[:, 8 * N:12 * N].rearrange("p (jq x) -> p jq x", jq=4),
                out_b[2][:, :].rearrange("(jq p) x -> p jq x", p=128),
            )
            for g in range(NG):
                scan_group(g, 8, 12)
        pos = {}
        for g in range(NG):
            scan_group(g, 12, 16)
            if g >= 1:
                memo_group(g - 1, pos[g - 1])
            pos[g] = omm_group(g)
        memo_group(NG - 1, pos[NG - 1])

        res = sm_p.tile([O, BL], f32)
        nc.vector.tensor_reduce(
            res[:, :],
            so_all[:, :].rearrange("p (t b) -> p b t", t=T),
            axis=mybir.AxisListType.X,
            op=Alu.add,
        )
        nc.sync.dma_start(out_d, res[:, :])

    _legalize_waits(nc, mybir)
    return nc


def _q8(a):
    import ml_dtypes
    return a.astype(ml_dtypes.float8_e4m3fn).astype(np.float32)


def _prep_inputs(x, W1, b1, Wo, bo, mode):
    import ml_dtypes
    f8 = ml_dtypes.float8_e4m3fn

    fp8 = mode == "fp8dr"
    x = np.ascontiguousarray(x, dtype=np.float32)
    xf = x.reshape(B, T, F)
    b1c = np.ascontiguousarray(b1.astype(np.float32).reshape(JT, 128).T)
    b1z = np.zeros_like(b1c)
    wot = np.ascontiguousarray(
        Wo.astype(np.float32).reshape(O, JT, 128).transpose(2, 1, 0).reshape(128, JT * O)
    )
    bo2 = np.ascontiguousarray(bo.astype(np.float32).reshape(O, 1))
    FH = F // 2

    wS = (W1.T.astype(np.float32) * np.float32(SW)).astype(np.float32)  # [F, HID]
    if fp8:
        wh_b = wS.astype(f8)
        wh = wh_b.astype(np.float32)
        w2_b = (_q8((wS - wh) * np.float32(16.0)) / np.float32(16.0)).astype(f8)
        w3_b = (wh / np.float32(4.0)).astype(f8)
        xS = (xf * np.float32(SX)).astype(np.float32)
        xh_b = xS.astype(f8)
        xh = xh_b.astype(np.float32)
        x2_b = (_q8((xS - xh) * np.float32(16.0)) / np.float32(4.0)).astype(f8)
        # per-core [F, N] transposed slices, as raw fp8
        xh_t = [np.ascontiguousarray(
            xh_b.reshape(B, T, F)[c * BL:(c + 1) * BL].transpose(2, 1, 0).reshape(F, N))
            for c in range(NCORES)]
        x2_t = [np.ascontiguousarray(
            x2_b.reshape(B, T, F)[c * BL:(c + 1) * BL].transpose(2, 1, 0).reshape(F, N))
            for c in range(NCORES)]
        # region-interleaved W rows: [KT, 3, 128, HID] -> [(k r p), h]
        w_all = np.stack([
            wh_b.reshape(KT, 128, HID),
            w2_b.reshape(KT, 128, HID),
            w3_b.reshape(KT, 128, HID),
        ], axis=1).reshape(KT * 3 * 128, HID)
    else:
        xts = [np.ascontiguousarray(
            xf[c * BL:(c + 1) * BL].transpose(2, 1, 0).reshape(F, N))
            for c in range(NCORES)]
        w1t = np.ascontiguousarray(W1.T, dtype=np.float32)

    in_maps = []
    for c in range(NCORES):
        lo = c & ~1
        half = c & 1
        kr = slice(half * FH, (half + 1) * FH)
        m = {
            "b1c": (b1c if half == 0 else b1z),
            "wot": wot, "bo2": bo2,
        }
        if fp8:
            krr = slice(half * (KH * 3 * 128), (half + 1) * (KH * 3 * 128))
            orr = slice((1 - half) * (KH * 3 * 128), (2 - half) * (KH * 3 * 128))
            m["xh8"] = np.ascontiguousarray(
                np.concatenate([xh_t[lo][kr], xh_t[lo + 1][kr]], axis=1))
            m["xl8"] = np.ascontiguousarray(
                np.concatenate([x2_t[lo][kr], x2_t[lo + 1][kr]], axis=1))
            m["w8"] = np.ascontiguousarray(w_all[krr])
            # own-batch full-F x for the last quad, k rows ordered
            # [own half, other half], packed 2 k-tiles per row
            okr = slice((1 - half) * FH, (2 - half) * FH)

            def pack_pairs(a):
                # [F, N] -> [(KT/2)*128, 2N]: row (kpair, p), cols [k0|k1]
                return np.ascontiguousarray(
                    a.reshape(KT // 2, 2, 128, N).transpose(0, 2, 1, 3)
                    .reshape((KT // 2) * 128, 2 * N))

            m["xq4h"] = pack_pairs(
                np.concatenate([xh_t[c][kr], xh_t[c][okr]], axis=0))
            m["xq4l"] = pack_pairs(
                np.concatenate([x2_t[c][kr], x2_t[c][okr]], axis=0))
            # other k-half of W, j12-15 columns only
            m["wq4"] = np.ascontiguousarray(w_all[orr, 3 * 512:4 * 512])
            m["b1f"] = b1c
        else:
            m["xt2b"] = np.ascontiguousarray(
                np.concatenate([xts[lo][kr], xts[lo + 1][kr]], axis=1))
            m["w1th"] = np.ascontiguousarray(w1t[kr])
        in_maps.append(m)
    return in_maps


def kernel(x, W1, b1, Wo, bo):
    from concourse import bass_utils

    mode = "fp8dr" if MM_MODE == "fp8dr" else "f32r"
    if "nc" not in _cache:
        _cache["nc"] = _build(mode)
    nc = _cache["nc"]

    in_maps = _prep_inputs(x, W1, b1, Wo, bo, mode)
    trace = os.environ.get("KERNEL_TRACE", "0") == "1"
    last_exc = None
    for _attempt in range(4):
        try:
            res = bass_utils.run_bass_kernel_spmd(
                nc, in_maps, core_ids=list(range(NCORES)), trace=trace
            )
            break
        except Exception as e:
            last_exc = e
            # transient device wedges (NRT_EXEC_UNIT_UNRECOVERABLE) usually
            # recover with a core reset + backoff
            os.environ["NEURON_RT_RESET_CORES"] = "1"
            import time
            time.sleep(5 * (_attempt + 1))
    else:
        raise last_exc
    if trace and res.exec_time_ns is not None:
        print(f"HW exec time: {res.exec_time_ns} ns")
        _cache["exec_time_ns"] = res.exec_time_ns

    out = np.empty((B, O), dtype=np.float32)
    for c in range(NCORES):
        out[c * BL:(c + 1) * BL, :] = res.results[c]["out"].T
    return out
